# revision 22
# baseline (speedup 1.0000x reference)
"""Luong attention (linear -> bmm -> mask -> softmax -> bmm) on 8 trn2 cores.

Reference (per batch b):
    q = h @ W.T + b                  [Tq, H]
    s = q @ x.T                      [Tq, Tk]
    s = where(mask, -inf, s)
    w = softmax(s, axis=-1)
    ctx = w @ x                      [Tq, H]

Sharding: pure data-parallel over B=16 -> 2 batches per core, no collectives.

Mask compaction: masked positions get softmax weight exactly 0, so the host
gathers only the unmasked rows of x (per batch), zero-padded to TKP (a
multiple of 128 covering the largest unmasked count across batches). Padded
columns carry an additive -1e30 mask row so exp() underflows to exactly 0,
and the padded x rows are zero so the context contribution is exactly 0.
This halves the score and context matmul work for ~50% dense masks while
remaining mathematically exact for ANY mask.

Per-core device pipeline (per batch):
    qT = Wt.T @ hT (+bias)           fp32r matmuls, qT kept [H, Tq] in SBUF
    per 128-row chunk of Tq:
      score psum = sum_k qT_k.T @ xT_k            fp32r, N-groups of <=256
      masked = score + padmask (DVE)              -> SBUF
      negmax = -rowmax(masked); w = exp(masked - max) with accum row sums
      wT = PE-transpose(w) in bf16 via PSUM
      ctx psum = sum_j wT_j.T @ x_j               bf16
      out = ctx * (1/rowsum)                      DVE scale, DMA out (gpsimd)

Host pre-layout: W.T tiled m-major pre-packed, hT tiled n-half-major (so the
first matmul group only needs ~2.5MB of DMA), compacted xT/x per batch, x
cast to bf16 for the context matmul, mask rows broadcast across partitions
by a 0-stride DMA. float32r (TF32-like) runs the PE at full rate for N>=256.
"""
import numpy as np

import concourse.bacc as bacc


def _install_ntff_hook_shim():
    """The agent image's `antenv` lacks `axon_hooks`; bass_utils imports it
    for trace=True under axon. Provide it and register the ctypes hook."""
    import sys
    import types
    try:
        import antenv.axon_hooks  # noqa: F401
        return
    except ImportError:
        pass
    mod = types.ModuleType("antenv.axon_hooks")
    _state = {"hook": None}
    mod.set_axon_ntff_profile_hook = lambda h: _state.__setitem__("hook", h)
    mod.get_axon_ntff_profile_hook = lambda: _state["hook"]
    sys.modules["antenv.axon_hooks"] = mod
    try:
        import antenv
        antenv.axon_hooks = mod
    except ImportError:
        pass
    try:
        from trn_agent_boot.trn_boot import _ntff_profile_via_ctypes
        hook = _ntff_profile_via_ctypes("/opt/axon/libaxon_pjrt.so")
        if hook is not None:
            mod.set_axon_ntff_profile_hook(hook)
    except Exception:
        pass


_install_ntff_hook_shim()

import concourse.mybir as mybir  # noqa: E402
import concourse.tile as tile  # noqa: E402
from concourse.bass_utils import run_bass_kernel_spmd  # noqa: E402

F32 = mybir.dt.float32
F32R = mybir.dt.float32r
BF16 = mybir.dt.bfloat16

B, TQ, TK, H = 16, 1024, 1024, 1024
NCORES = 8
BPC = B // NCORES          # batches per core
P = 128
KT = H // P                # 8 k-tiles of the contraction dims
CH = TQ // P               # 8 q-row chunks per batch
NH = TQ // 512             # q-phase free-dim halves

_MASK_NEG = np.float32(-1e30)


def _score_groups(tkp):
    """Bank-packed group sizes: one group per PSUM bank, each >=256 where
    possible (avoids any small-N fp32r edge) and never spanning a bank."""
    rem, groups = tkp, []
    while rem > 0:
        if rem > 512:
            g = 512 if rem - 512 >= 256 else rem - 256
        else:
            g = rem
        groups.append(g)
        rem -= g
    return groups


def _build_nc(tkp0, tkp1):
    tkpm = max(tkp0, tkp1)     # DRAM tensors sized to the wider slot
    nc = bacc.Bacc("TRN2", target_bir_lowering=False)
    # Wm: [m, 128, H] m-major column tiles of W.T, pre-packed [p, k, c]
    Wm_d = nc.dram_tensor("Wm", [KT, P, H], F32R, kind="ExternalInput")
    # hT tiled [b, n, k, 128, 512]
    hT_d = nc.dram_tensor("hT", [BPC, NH, KT, P, 512], F32R, kind="ExternalInput")
    xT_d = nc.dram_tensor("xT", [BPC, H, tkpm], F32R, kind="ExternalInput")
    xn_d = nc.dram_tensor("xn", [BPC, tkpm, H], BF16, kind="ExternalInput")
    bias_d = nc.dram_tensor("bias", [H], F32, kind="ExternalInput")
    am_d = nc.dram_tensor("amask", [BPC, tkpm], F32, kind="ExternalInput")
    ctx_d = nc.dram_tensor("ctx", [BPC, TQ, H], F32, kind="ExternalOutput")

    with tile.TileContext(nc) as tc:
        with (
            tc.tile_pool(name="consts", bufs=1) as consts,
            tc.tile_pool(name="hTp", bufs=2) as hT_pool,
            tc.tile_pool(name="qTp", bufs=1) as qT_pool,
            tc.tile_pool(name="xTp", bufs=1) as xT_pool,
            tc.tile_pool(name="xnp", bufs=1) as xn_pool,
            tc.tile_pool(name="chk", bufs=2) as chk,
            tc.tile_pool(name="stat", bufs=2) as stat,
            tc.tile_pool(name="ps_s", bufs=2, space="PSUM") as ps_s,
            tc.tile_pool(name="ps_w", bufs=1, space="PSUM") as ps_w,
            tc.tile_pool(name="ps_c", bufs=1, space="PSUM") as ps_c,
        ):
            bias_sb = consts.tile([P, KT], F32, tag="bias")
            from concourse.masks import make_identity
            ident = consts.tile([P, P], BF16, tag="ident")
            make_identity(nc, ident)
            w_sb = consts.tile([P, KT, KT, P], F32R, tag="W")  # [p, m, k, c]

            def dma_w(m):
                nc.scalar.dma_start(
                    w_sb[:, m],
                    Wm_d[m].rearrange("p (k c) -> p k c", k=KT),
                )

            for b in range(BPC):
                tkp = (tkp0, tkp1)[b]   # per-slot compacted width
                jt = tkp // P
                # ---- input DMAs, first-needed-first ----
                hT_sb = hT_pool.tile([P, KT, TQ], F32R, tag="hT")
                if b == 0:
                    dma_w(0)
                for n in range(NH):
                    for k in range(KT):
                        nc.sync.dma_start(
                            hT_sb[:, k, n * 512:(n + 1) * 512], hT_d[b, n, k]
                        )
                    if b == 0 and n == 0:
                        for m in range(1, KT):
                            dma_w(m)
                        nc.sync.dma_start(
                            bias_sb,
                            bias_d[:].rearrange("(t p) -> p t", p=P))
                mask_sb = stat.tile([P, tkp], F32, tag="maskb")
                nc.sync.dma_start(
                    mask_sb, am_d[b:b + 1, 0:tkp].partition_broadcast(P)
                )

                # ---- q projection: qT[m][o, t] = sum_k Wm_k.T @ hT_k ----
                qT_sb = qT_pool.tile([P, KT, TQ], F32R, tag="qT")
                for n in range(NH):
                    for m in range(KT):
                        qp = ps_s.tile([P, 512], F32, tag="sp")
                        for k in range(KT):
                            nc.tensor.matmul(
                                qp,
                                w_sb[:, m, k],
                                hT_sb[:, k, n * 512:(n + 1) * 512],
                                start=(k == 0),
                                stop=(k == KT - 1),
                            )
                        nc.vector.tensor_scalar_add(
                            qT_sb[:, m, n * 512:(n + 1) * 512], qp,
                            bias_sb[:, m:m + 1],
                        )

                # xT/xn after q emission: needed only by the chunk phase.
                xT_sb = xT_pool.tile([P, KT, tkp], F32R, tag="xT")
                for k in range(KT):
                    nc.sync.dma_start(
                        xT_sb[:, k], xT_d[b, k * P:(k + 1) * P, 0:tkp])
                xn_sb = xn_pool.tile([P, jt, H], BF16, tag="xn")
                for j in range(jt):
                    nc.sync.dma_start(xn_sb[:, j], xn_d[b, j * P:(j + 1) * P, :])

                # ---- chunk pipeline over Tq rows ----
                carry = None
                for c in range(CH + 1):
                    if carry is not None:
                        pc, w_t, rsum_t = carry
                        wTp = ps_w.tile([P, tkp], BF16, tag="wt")
                        for j in range(jt):
                            nc.tensor.transpose(
                                wTp[:, j * P:(j + 1) * P],
                                w_t[:, j * P:(j + 1) * P],
                                ident,
                            )
                        wT_sb = chk.tile([P, tkp], BF16, tag="wT")
                        nc.vector.tensor_copy(wT_sb, wTp)

                    if c < CH:
                        groups = _score_groups(tkp)
                        sp = ps_s.tile([P, len(groups), 512], F32, tag="sp")
                        gs = 0
                        for gi, gn in enumerate(groups):
                            for k in range(KT):
                                nc.tensor.matmul(
                                    sp[:, gi, 0:gn],
                                    qT_sb[:, k, c * P:(c + 1) * P],
                                    xT_sb[:, k, gs:gs + gn],
                                    start=(k == 0),
                                    stop=(k == KT - 1),
                                )
                            gs += gn
                        sc_sb = chk.tile([P, tkp], F32, tag="sc")
                        gs = 0
                        for gi, gn in enumerate(groups):
                            nc.vector.tensor_add(
                                sc_sb[:, gs:gs + gn],
                                sp[:, gi, 0:gn],
                                mask_sb[:, gs:gs + gn],
                            )
                            gs += gn
                        negmax = stat.tile([P, 1], F32, tag="negmax")
                        nc.vector.reduce_max(
                            negmax, sc_sb, axis=mybir.AxisListType.X, negate=True
                        )
                        w_new = chk.tile([P, tkp], BF16, tag="w")
                        ssum = stat.tile([P, 1], F32, tag="ssum")
                        nc.scalar.activation(
                            w_new, sc_sb, mybir.ActivationFunctionType.Exp,
                            bias=negmax, scale=1.0, accum_out=ssum,
                        )
                        rsum_new = stat.tile([P, 1], F32, tag="rsum")
                        nc.vector.reciprocal(rsum_new, ssum)

                    if carry is not None:
                        pc, w_t, rsum_t = carry
                        cxp = ps_c.tile([P, H], F32, tag="cx")
                        for n in range(H // 512):
                            for j in range(jt):
                                nc.tensor.matmul(
                                    cxp[:, n * 512:(n + 1) * 512],
                                    wT_sb[:, j * P:(j + 1) * P],
                                    xn_sb[:, j, n * 512:(n + 1) * 512],
                                    start=(j == 0),
                                    stop=(j == jt - 1),
                                )
                        outc = chk.tile([P, H], F32, tag="outc")
                        for n in range(H // 512):
                            nsl = slice(n * 512, (n + 1) * 512)
                            nc.vector.tensor_scalar_mul(
                                outc[:, nsl], cxp[:, nsl], rsum_t
                            )
                            nc.gpsimd.dma_start(
                                ctx_d[b, pc * P:(pc + 1) * P, nsl],
                                outc[:, nsl],
                            )

                    carry = (c, w_new, rsum_new) if c < CH else None
    return nc


_CACHE = {}


def _get_nc(tkp0, tkp1):
    key = (tkp0, tkp1)
    if key not in _CACHE:
        nc = _build_nc(tkp0, tkp1)
        nc.compile()
        _CACHE[key] = nc
    return _CACHE[key]


def kernel(h_t_dec, x_enc, mask, W, b, _trace=False, _trace_kwargs=None):
    import ml_dtypes

    h_t_dec = np.ascontiguousarray(h_t_dec, dtype=np.float32)
    x_enc = np.ascontiguousarray(x_enc, dtype=np.float32)
    mask = np.asarray(mask).astype(bool)
    W = np.ascontiguousarray(W, dtype=np.float32)
    b = np.ascontiguousarray(b, dtype=np.float32)

    Wt = np.ascontiguousarray(W.T)                      # [H_in, H_out]
    # Wm[m, p, k, c] = Wt[k*128 + p, m*128 + c] -> contiguous per-m DMA
    Wm = np.ascontiguousarray(
        Wt.reshape(KT, P, KT, P).transpose(2, 1, 0, 3).reshape(KT, P, H))

    keep = [np.nonzero(~mask[bi])[0] for bi in range(B)]

    def pad128(n):
        return min(TK, max(P, ((n + P - 1) // P) * P))

    # Load-balance: sort batches by unmasked count; slot 0 takes the 8
    # smallest, slot 1 the 8 largest, so each slot's compiled width covers
    # only its own worst case instead of the global max.
    order = np.argsort([len(k) for k in keep], kind="stable")
    slot_batches = [order[:NCORES], order[NCORES:]]        # [slot][core]
    tkp0 = pad128(max(len(keep[g]) for g in slot_batches[0]))
    tkp1 = pad128(max(len(keep[g]) for g in slot_batches[1]))
    tkpm = max(tkp0, tkp1)

    # compacted x at the max width: unmasked rows first, zero rows beyond
    xc = np.zeros((B, tkpm, H), dtype=np.float32)
    amask_full = np.full((B, tkpm), _MASK_NEG, dtype=np.float32)
    for bi in range(B):
        nk = len(keep[bi])
        xc[bi, :nk] = x_enc[bi, keep[bi]]
        amask_full[bi, :nk] = 0.0

    in_maps = []
    for core in range(NCORES):
        gb = [slot_batches[0][core], slot_batches[1][core]]
        hT = h_t_dec[gb].transpose(0, 2, 1)              # [b, H, Tq]
        hT_t = np.ascontiguousarray(
            hT.reshape(BPC, KT, P, NH, 512).transpose(0, 3, 1, 2, 4))
        xT = np.ascontiguousarray(xc[gb].transpose(0, 2, 1))
        xn = np.ascontiguousarray(xc[gb]).astype(ml_dtypes.bfloat16)
        in_maps.append({
            "hT": hT_t,
            "xT": xT,
            "xn": xn,
            "Wm": Wm,
            "bias": b,
            "amask": np.ascontiguousarray(amask_full[gb]),
        })

    nc = _get_nc(tkp0, tkp1)
    res = run_bass_kernel_spmd(
        nc, in_maps, core_ids=list(range(NCORES)),
        trace=_trace, trace_kwargs=_trace_kwargs or {},
    )
    out = np.empty((B, TQ, H), dtype=np.float32)
    for core in range(NCORES):
        out[slot_batches[0][core]] = res.results[core]["ctx"][0]
        out[slot_batches[1][core]] = res.results[core]["ctx"][1]
    if _trace:
        return out, res
    return out


# revision 25
# speedup vs baseline: 1.0127x; 1.0127x over previous
"""Luong attention (linear -> bmm -> mask -> softmax -> bmm) on 8 trn2 cores.

Reference (per batch b):
    q = h @ W.T + b                  [Tq, H]
    s = q @ x.T                      [Tq, Tk]
    s = where(mask, -inf, s)
    w = softmax(s, axis=-1)
    ctx = w @ x                      [Tq, H]

Sharding: pure data-parallel over B=16 -> 2 batches per core, no collectives.

Mask compaction: masked positions get softmax weight exactly 0, so the host
gathers only the unmasked rows of x (per batch), zero-padded to TKP (a
multiple of 128 covering the largest unmasked count across batches). Padded
columns carry an additive -1e30 mask row so exp() underflows to exactly 0,
and the padded x rows are zero so the context contribution is exactly 0.
This halves the score and context matmul work for ~50% dense masks while
remaining mathematically exact for ANY mask.

Per-core device pipeline (per batch):
    qT = Wt.T @ hT (+bias)           fp32r matmuls, qT kept [H, Tq] in SBUF
    per 128-row chunk of Tq:
      score psum = sum_k qT_k.T @ xT_k            fp32r, N-groups of <=256
      masked = score + padmask (DVE)              -> SBUF
      negmax = -rowmax(masked); w = exp(masked - max) with accum row sums
      wT = PE-transpose(w) in bf16 via PSUM
      ctx psum = sum_j wT_j.T @ x_j               bf16
      out = ctx * (1/rowsum)                      DVE scale, DMA out (gpsimd)

Host pre-layout: W.T tiled m-major pre-packed, hT tiled n-half-major (so the
first matmul group only needs ~2.5MB of DMA), compacted xT/x per batch, x
cast to bf16 for the context matmul, mask rows broadcast across partitions
by a 0-stride DMA. float32r (TF32-like) runs the PE at full rate for N>=256.
"""
import numpy as np

import concourse.bacc as bacc


def _install_ntff_hook_shim():
    """The agent image's `antenv` lacks `axon_hooks`; bass_utils imports it
    for trace=True under axon. Provide it and register the ctypes hook."""
    import sys
    import types
    try:
        import antenv.axon_hooks  # noqa: F401
        return
    except ImportError:
        pass
    mod = types.ModuleType("antenv.axon_hooks")
    _state = {"hook": None}
    mod.set_axon_ntff_profile_hook = lambda h: _state.__setitem__("hook", h)
    mod.get_axon_ntff_profile_hook = lambda: _state["hook"]
    sys.modules["antenv.axon_hooks"] = mod
    try:
        import antenv
        antenv.axon_hooks = mod
    except ImportError:
        pass
    try:
        from trn_agent_boot.trn_boot import _ntff_profile_via_ctypes
        hook = _ntff_profile_via_ctypes("/opt/axon/libaxon_pjrt.so")
        if hook is not None:
            mod.set_axon_ntff_profile_hook(hook)
    except Exception:
        pass


_install_ntff_hook_shim()

import concourse.mybir as mybir  # noqa: E402
import concourse.tile as tile  # noqa: E402
from concourse.bass_utils import run_bass_kernel_spmd  # noqa: E402

F32 = mybir.dt.float32
F32R = mybir.dt.float32r
BF16 = mybir.dt.bfloat16

B, TQ, TK, H = 16, 1024, 1024, 1024
NCORES = 8
BPC = B // NCORES          # batches per core
P = 128
KT = H // P                # 8 k-tiles of the contraction dims
CH = TQ // P               # 8 q-row chunks per batch
NH = TQ // 512             # q-phase free-dim halves

_MASK_NEG = np.float32(-1e30)


def _score_groups(tkp):
    """Bank-packed group sizes: one group per PSUM bank, each >=256 where
    possible (avoids any small-N fp32r edge) and never spanning a bank."""
    rem, groups = tkp, []
    while rem > 0:
        if rem > 512:
            g = 512 if rem - 512 >= 256 else rem - 256
        else:
            g = rem
        groups.append(g)
        rem -= g
    return groups


def _build_nc(tkp0, tkp1):
    tkpm = max(tkp0, tkp1)     # DRAM tensors sized to the wider slot
    nc = bacc.Bacc("TRN2", target_bir_lowering=False)
    # Wm: [m, 128, H] m-major column tiles of W.T, pre-packed [p, k, c]
    Wm_d = nc.dram_tensor("Wm", [KT, P, H], F32R, kind="ExternalInput")
    # hT tiled [b, n, k, 128, 512]
    hT_d = nc.dram_tensor("hT", [BPC, NH, KT, P, 512], F32R, kind="ExternalInput")
    xT_d = nc.dram_tensor("xT", [BPC, H, tkpm], F32R, kind="ExternalInput")
    xn_d = nc.dram_tensor("xn", [BPC, tkpm, H], BF16, kind="ExternalInput")
    bias_d = nc.dram_tensor("bias", [H], F32, kind="ExternalInput")
    am_d = nc.dram_tensor("amask", [BPC, tkpm], F32, kind="ExternalInput")
    ctx_d = nc.dram_tensor("ctx", [BPC, TQ, H], F32, kind="ExternalOutput")

    with tile.TileContext(nc) as tc:
        with (
            tc.tile_pool(name="consts", bufs=1) as consts,
            tc.tile_pool(name="hTp", bufs=2) as hT_pool,
            tc.tile_pool(name="qTp", bufs=1) as qT_pool,
            tc.tile_pool(name="xTp", bufs=1) as xT_pool,
            tc.tile_pool(name="xnp", bufs=1) as xn_pool,
            tc.tile_pool(name="chk", bufs=2) as chk,
            tc.tile_pool(name="stat", bufs=2) as stat,
            tc.tile_pool(name="ps_s", bufs=2, space="PSUM") as ps_s,
            tc.tile_pool(name="ps_w", bufs=1, space="PSUM") as ps_w,
            tc.tile_pool(name="ps_c", bufs=1, space="PSUM") as ps_c,
        ):
            bias_sb = consts.tile([P, KT], F32, tag="bias")
            from concourse.masks import make_identity
            ident = consts.tile([P, P], BF16, tag="ident")
            make_identity(nc, ident)
            w_sb = consts.tile([P, KT, KT, P], F32R, tag="W")  # [p, m, k, c]

            def dma_w(m):
                nc.scalar.dma_start(
                    w_sb[:, m],
                    Wm_d[m].rearrange("p (k c) -> p k c", k=KT),
                )

            def emit_tail(tb, ttkp, tjt, txn_sb, tcarry):
                pc, w_t, rsum_t = tcarry
                wTp = ps_w.tile([P, ttkp], BF16, tag="wt")
                for j in range(tjt):
                    nc.tensor.transpose(
                        wTp[:, j * P:(j + 1) * P],
                        w_t[:, j * P:(j + 1) * P],
                        ident,
                    )
                wT_sb = chk.tile([P, ttkp], BF16, tag="wT")
                nc.vector.tensor_copy(wT_sb, wTp)
                cxp = ps_c.tile([P, H], F32, tag="cx")
                for n in range(H // 512):
                    for j in range(tjt):
                        nc.tensor.matmul(
                            cxp[:, n * 512:(n + 1) * 512],
                            wT_sb[:, j * P:(j + 1) * P],
                            txn_sb[:, j, n * 512:(n + 1) * 512],
                            start=(j == 0),
                            stop=(j == tjt - 1),
                        )
                outc = chk.tile([P, H], F32, tag="outc")
                for n in range(H // 512):
                    nsl = slice(n * 512, (n + 1) * 512)
                    nc.vector.tensor_scalar_mul(
                        outc[:, nsl], cxp[:, nsl], rsum_t
                    )
                    nc.gpsimd.dma_start(
                        ctx_d[tb, pc * P:(pc + 1) * P, nsl],
                        outc[:, nsl],
                    )

            pending = None
            for b in range(BPC):
                tkp = (tkp0, tkp1)[b]   # per-slot compacted width
                jt = tkp // P
                # ---- input DMAs, first-needed-first ----
                hT_sb = hT_pool.tile([P, KT, TQ], F32R, tag="hT")
                if b == 0:
                    dma_w(0)
                for n in range(NH):
                    for k in range(KT):
                        nc.sync.dma_start(
                            hT_sb[:, k, n * 512:(n + 1) * 512], hT_d[b, n, k]
                        )
                    if b == 0 and n == 0:
                        for m in range(1, KT):
                            dma_w(m)
                        nc.sync.dma_start(
                            bias_sb,
                            bias_d[:].rearrange("(t p) -> p t", p=P))
                mask_sb = stat.tile([P, tkp], F32, tag="maskb")
                nc.sync.dma_start(
                    mask_sb, am_d[b:b + 1, 0:tkp].partition_broadcast(P)
                )

                # ---- q projection: qT[m][o, t] = sum_k Wm_k.T @ hT_k ----
                qT_sb = qT_pool.tile([P, KT, TQ], F32R, tag="qT")
                for n in range(NH):
                    for m in range(KT):
                        qp = ps_s.tile([P, 512], F32, tag="sp")
                        for k in range(KT):
                            nc.tensor.matmul(
                                qp,
                                w_sb[:, m, k],
                                hT_sb[:, k, n * 512:(n + 1) * 512],
                                start=(k == 0),
                                stop=(k == KT - 1),
                            )
                        nc.vector.tensor_scalar_add(
                            qT_sb[:, m, n * 512:(n + 1) * 512], qp,
                            bias_sb[:, m:m + 1],
                        )

                # xT/xn after q emission: needed only by the chunk phase.
                xT_sb = xT_pool.tile([P, KT, tkp], F32R, tag="xT")
                for k in range(KT):
                    nc.sync.dma_start(
                        xT_sb[:, k], xT_d[b, k * P:(k + 1) * P, 0:tkp])
                xn_sb = xn_pool.tile([P, jt, H], BF16, tag="xn")
                for j in range(jt):
                    nc.sync.dma_start(xn_sb[:, j], xn_d[b, j * P:(j + 1) * P, :])

                # ---- deferred tail of the previous batch: emitted here
                # so this batch's q matmuls hide its softmax latency ----
                if pending is not None:
                    emit_tail(*pending)
                    pending = None

                # ---- chunk pipeline over Tq rows ----
                carry = None
                for c in range(CH + 1):
                    if c == CH:
                        pending = (b, tkp, jt, xn_sb, carry)
                        carry = None
                        break
                    if carry is not None:
                        pc, w_t, rsum_t = carry
                        wTp = ps_w.tile([P, tkp], BF16, tag="wt")
                        for j in range(jt):
                            nc.tensor.transpose(
                                wTp[:, j * P:(j + 1) * P],
                                w_t[:, j * P:(j + 1) * P],
                                ident,
                            )
                        wT_sb = chk.tile([P, tkp], BF16, tag="wT")
                        nc.vector.tensor_copy(wT_sb, wTp)

                    if c < CH:
                        groups = _score_groups(tkp)
                        sp = ps_s.tile([P, len(groups), 512], F32, tag="sp")
                        gs = 0
                        for gi, gn in enumerate(groups):
                            for k in range(KT):
                                nc.tensor.matmul(
                                    sp[:, gi, 0:gn],
                                    qT_sb[:, k, c * P:(c + 1) * P],
                                    xT_sb[:, k, gs:gs + gn],
                                    start=(k == 0),
                                    stop=(k == KT - 1),
                                )
                            gs += gn
                        sc_sb = chk.tile([P, tkp], F32, tag="sc")
                        gs = 0
                        for gi, gn in enumerate(groups):
                            nc.vector.tensor_add(
                                sc_sb[:, gs:gs + gn],
                                sp[:, gi, 0:gn],
                                mask_sb[:, gs:gs + gn],
                            )
                            gs += gn
                        negmax = stat.tile([P, 1], F32, tag="negmax")
                        nc.vector.reduce_max(
                            negmax, sc_sb, axis=mybir.AxisListType.X, negate=True
                        )
                        w_new = chk.tile([P, tkp], BF16, tag="w")
                        ssum = stat.tile([P, 1], F32, tag="ssum")
                        nc.scalar.activation(
                            w_new, sc_sb, mybir.ActivationFunctionType.Exp,
                            bias=negmax, scale=1.0, accum_out=ssum,
                        )
                        rsum_new = stat.tile([P, 1], F32, tag="rsum")
                        nc.vector.reciprocal(rsum_new, ssum)

                    if carry is not None:
                        pc, w_t, rsum_t = carry
                        cxp = ps_c.tile([P, H], F32, tag="cx")
                        for n in range(H // 512):
                            for j in range(jt):
                                nc.tensor.matmul(
                                    cxp[:, n * 512:(n + 1) * 512],
                                    wT_sb[:, j * P:(j + 1) * P],
                                    xn_sb[:, j, n * 512:(n + 1) * 512],
                                    start=(j == 0),
                                    stop=(j == jt - 1),
                                )
                        outc = chk.tile([P, H], F32, tag="outc")
                        for n in range(H // 512):
                            nsl = slice(n * 512, (n + 1) * 512)
                            nc.vector.tensor_scalar_mul(
                                outc[:, nsl], cxp[:, nsl], rsum_t
                            )
                            nc.gpsimd.dma_start(
                                ctx_d[b, pc * P:(pc + 1) * P, nsl],
                                outc[:, nsl],
                            )

                    carry = (c, w_new, rsum_new) if c < CH else None

            if pending is not None:
                emit_tail(*pending)
    return nc


_CACHE = {}


def _get_nc(tkp0, tkp1):
    key = (tkp0, tkp1)
    if key not in _CACHE:
        nc = _build_nc(tkp0, tkp1)
        nc.compile()
        _CACHE[key] = nc
    return _CACHE[key]


def kernel(h_t_dec, x_enc, mask, W, b, _trace=False, _trace_kwargs=None):
    import ml_dtypes

    h_t_dec = np.ascontiguousarray(h_t_dec, dtype=np.float32)
    x_enc = np.ascontiguousarray(x_enc, dtype=np.float32)
    mask = np.asarray(mask).astype(bool)
    W = np.ascontiguousarray(W, dtype=np.float32)
    b = np.ascontiguousarray(b, dtype=np.float32)

    Wt = np.ascontiguousarray(W.T)                      # [H_in, H_out]
    # Wm[m, p, k, c] = Wt[k*128 + p, m*128 + c] -> contiguous per-m DMA
    Wm = np.ascontiguousarray(
        Wt.reshape(KT, P, KT, P).transpose(2, 1, 0, 3).reshape(KT, P, H))

    keep = [np.nonzero(~mask[bi])[0] for bi in range(B)]

    def pad128(n):
        return min(TK, max(P, ((n + P - 1) // P) * P))

    # Load-balance: sort batches by unmasked count; slot 0 takes the 8
    # smallest, slot 1 the 8 largest, so each slot's compiled width covers
    # only its own worst case instead of the global max.
    order = np.argsort([len(k) for k in keep], kind="stable")
    slot_batches = [order[:NCORES], order[NCORES:]]        # [slot][core]
    tkp0 = pad128(max(len(keep[g]) for g in slot_batches[0]))
    tkp1 = pad128(max(len(keep[g]) for g in slot_batches[1]))
    tkpm = max(tkp0, tkp1)

    # compacted x at the max width: unmasked rows first, zero rows beyond
    xc = np.zeros((B, tkpm, H), dtype=np.float32)
    amask_full = np.full((B, tkpm), _MASK_NEG, dtype=np.float32)
    for bi in range(B):
        nk = len(keep[bi])
        xc[bi, :nk] = x_enc[bi, keep[bi]]
        amask_full[bi, :nk] = 0.0

    in_maps = []
    for core in range(NCORES):
        gb = [slot_batches[0][core], slot_batches[1][core]]
        hT = h_t_dec[gb].transpose(0, 2, 1)              # [b, H, Tq]
        hT_t = np.ascontiguousarray(
            hT.reshape(BPC, KT, P, NH, 512).transpose(0, 3, 1, 2, 4))
        xT = np.ascontiguousarray(xc[gb].transpose(0, 2, 1))
        xn = np.ascontiguousarray(xc[gb]).astype(ml_dtypes.bfloat16)
        in_maps.append({
            "hT": hT_t,
            "xT": xT,
            "xn": xn,
            "Wm": Wm,
            "bias": b,
            "amask": np.ascontiguousarray(amask_full[gb]),
        })

    nc = _get_nc(tkp0, tkp1)
    res = run_bass_kernel_spmd(
        nc, in_maps, core_ids=list(range(NCORES)),
        trace=_trace, trace_kwargs=_trace_kwargs or {},
    )
    out = np.empty((B, TQ, H), dtype=np.float32)
    for core in range(NCORES):
        out[slot_batches[0][core]] = res.results[core]["ctx"][0]
        out[slot_batches[1][core]] = res.results[core]["ctx"][1]
    if _trace:
        return out, res
    return out


# revision 27
# speedup vs baseline: 1.1522x; 1.1378x over previous
"""Luong attention (linear -> bmm -> mask -> softmax -> bmm) on 8 trn2 cores.

Reference (per batch b):
    q = h @ W.T + b                  [Tq, H]
    s = q @ x.T                      [Tq, Tk]
    s = where(mask, -inf, s)
    w = softmax(s, axis=-1)
    ctx = w @ x                      [Tq, H]

Sharding: pure data-parallel over B=16 -> 2 batches per core, no collectives.

Mask compaction: masked positions get softmax weight exactly 0, so the host
gathers only the unmasked rows of x (per batch), zero-padded to TKP (a
multiple of 128 covering the largest unmasked count across batches). Padded
columns carry an additive -1e30 mask row so exp() underflows to exactly 0,
and the padded x rows are zero so the context contribution is exactly 0.
This halves the score and context matmul work for ~50% dense masks while
remaining mathematically exact for ANY mask.

Per-core device pipeline (per batch):
    qT = Wt.T @ hT (+bias)           fp32r matmuls, qT kept [H, Tq] in SBUF
    per 128-row chunk of Tq:
      score psum = sum_k qT_k.T @ xT_k            fp32r, N-groups of <=256
      masked = score + padmask (DVE)              -> SBUF
      negmax = -rowmax(masked); w = exp(masked - max) with accum row sums
      wT = PE-transpose(w) in bf16 via PSUM
      ctx psum = sum_j wT_j.T @ x_j               bf16
      out = ctx * (1/rowsum)                      DVE scale, DMA out (gpsimd)

Host pre-layout: W.T tiled m-major pre-packed, hT tiled n-half-major (so the
first matmul group only needs ~2.5MB of DMA), compacted xT/x per batch, x
cast to bf16 for the context matmul, mask rows broadcast across partitions
by a 0-stride DMA. float32r (TF32-like) runs the PE at full rate for N>=256.
"""
import numpy as np

import concourse.bacc as bacc


def _install_ntff_hook_shim():
    """The agent image's `antenv` lacks `axon_hooks`; bass_utils imports it
    for trace=True under axon. Provide it and register the ctypes hook."""
    import sys
    import types
    try:
        import antenv.axon_hooks  # noqa: F401
        return
    except ImportError:
        pass
    mod = types.ModuleType("antenv.axon_hooks")
    _state = {"hook": None}
    mod.set_axon_ntff_profile_hook = lambda h: _state.__setitem__("hook", h)
    mod.get_axon_ntff_profile_hook = lambda: _state["hook"]
    sys.modules["antenv.axon_hooks"] = mod
    try:
        import antenv
        antenv.axon_hooks = mod
    except ImportError:
        pass
    try:
        from trn_agent_boot.trn_boot import _ntff_profile_via_ctypes
        hook = _ntff_profile_via_ctypes("/opt/axon/libaxon_pjrt.so")
        if hook is not None:
            mod.set_axon_ntff_profile_hook(hook)
    except Exception:
        pass


_install_ntff_hook_shim()

import concourse.mybir as mybir  # noqa: E402
import concourse.tile as tile  # noqa: E402
from concourse.bass_utils import run_bass_kernel_spmd  # noqa: E402

F32 = mybir.dt.float32
F32R = mybir.dt.float32r
BF16 = mybir.dt.bfloat16

B, TQ, TK, H = 16, 1024, 1024, 1024
NCORES = 8
BPC = B // NCORES          # batches per core
P = 128
KT = H // P                # 8 k-tiles of the contraction dims
CH = TQ // P               # 8 q-row chunks per batch
NH = TQ // 512             # q-phase free-dim halves

_MASK_NEG = np.float32(-1e30)


def _score_groups(tkp):
    """Bank-packed group sizes: one group per PSUM bank, each >=256 where
    possible (avoids any small-N fp32r edge) and never spanning a bank."""
    rem, groups = tkp, []
    while rem > 0:
        if rem > 512:
            g = 512 if rem - 512 >= 256 else rem - 256
        else:
            g = rem
        groups.append(g)
        rem -= g
    return groups


def _build_nc(tkp0, tkp1):
    tkpm = max(tkp0, tkp1)     # DRAM tensors sized to the wider slot
    nc = bacc.Bacc("TRN2", target_bir_lowering=False)
    # Wm: [m, 128, H] m-major column tiles of W.T, pre-packed [p, k, c]
    Wm_d = nc.dram_tensor("Wm", [KT, P, H], F32R, kind="ExternalInput")
    # hT tiled [b, n, k, 128, 512]
    hT_d = nc.dram_tensor("hT", [BPC, NH, KT, P, 512], F32R, kind="ExternalInput")
    xT_d = nc.dram_tensor("xT", [BPC, H, tkpm], F32R, kind="ExternalInput")
    xn_d = nc.dram_tensor("xn", [BPC, tkpm, H], BF16, kind="ExternalInput")
    am_d = nc.dram_tensor("amask", [BPC, tkpm], F32, kind="ExternalInput")
    ctx_d = nc.dram_tensor("ctx", [BPC, TQ, H], F32, kind="ExternalOutput")

    with tile.TileContext(nc) as tc:
        with (
            tc.tile_pool(name="consts", bufs=1) as consts,
            tc.tile_pool(name="hTp", bufs=2) as hT_pool,
            tc.tile_pool(name="qTp", bufs=1) as qT_pool,
            tc.tile_pool(name="xTp", bufs=1) as xT_pool,
            tc.tile_pool(name="xnp", bufs=1) as xn_pool,
            tc.tile_pool(name="chk", bufs=2) as chk,
            tc.tile_pool(name="stat", bufs=2) as stat,
            tc.tile_pool(name="ps_s", bufs=2, space="PSUM") as ps_s,
            tc.tile_pool(name="ps_w", bufs=1, space="PSUM") as ps_w,
            tc.tile_pool(name="ps_c", bufs=1, space="PSUM") as ps_c,
        ):
            from concourse.masks import make_identity
            ident = consts.tile([P, P], BF16, tag="ident")
            make_identity(nc, ident)
            w_sb = consts.tile([P, KT, KT, P], F32R, tag="W")  # [p, m, k, c]

            def dma_w(m):
                nc.scalar.dma_start(
                    w_sb[:, m],
                    Wm_d[m].rearrange("p (k c) -> p k c", k=KT),
                )

            for b in range(BPC):
                tkp = (tkp0, tkp1)[b]   # per-slot compacted width
                jt = tkp // P
                # ---- input DMAs, first-needed-first: the projection
                # z = x@W needs xT + W; hT is only the score stationary ----
                xT_sb = xT_pool.tile([P, KT, tkp], F32R, tag="xT")
                for k in range(KT):
                    nc.sync.dma_start(
                        xT_sb[:, k], xT_d[b, k * P:(k + 1) * P, 0:tkp])
                if b == 0:
                    for m in range(KT):
                        dma_w(m)
                hT_sb = hT_pool.tile([P, KT, TQ], F32R, tag="hT")
                for n in range(NH):
                    for k in range(KT):
                        nc.sync.dma_start(
                            hT_sb[:, k, n * 512:(n + 1) * 512], hT_d[b, n, k]
                        )
                mask_sb = stat.tile([P, tkp], F32, tag="maskb")
                nc.sync.dma_start(
                    mask_sb, am_d[b:b + 1, 0:tkp].partition_broadcast(P)
                )
                xn_sb = xn_pool.tile([P, jt, H], BF16, tag="xn")
                for j in range(jt):
                    nc.sync.dma_start(xn_sb[:, j], xn_d[b, j * P:(j + 1) * P, :])

                # ---- projection over the COMPACTED width:
                # zT[m][i, s] = sum_k Wn_k,m.T @ xT_k  (z = x @ W) ----
                zT_sb = qT_pool.tile([P, KT, tkp], F32R, tag="qT")
                for m in range(KT):
                    gs = 0
                    for gn in _score_groups(tkp):
                        zp = ps_s.tile([P, 512], F32, tag="sp")
                        for k in range(KT):
                            nc.tensor.matmul(
                                zp[:, 0:gn],
                                w_sb[:, m, k],
                                xT_sb[:, k, gs:gs + gn],
                                start=(k == 0),
                                stop=(k == KT - 1),
                            )
                        nc.vector.tensor_copy(
                            zT_sb[:, m, gs:gs + gn], zp[:, 0:gn])
                        gs += gn

                # ---- chunk pipeline over Tq rows ----
                carry = None
                for c in range(CH + 1):
                    if carry is not None:
                        pc, w_t, rsum_t = carry
                        wTp = ps_w.tile([P, tkp], BF16, tag="wt")
                        for j in range(jt):
                            nc.tensor.transpose(
                                wTp[:, j * P:(j + 1) * P],
                                w_t[:, j * P:(j + 1) * P],
                                ident,
                            )
                        wT_sb = chk.tile([P, tkp], BF16, tag="wT")
                        nc.vector.tensor_copy(wT_sb, wTp)

                    if c < CH:
                        groups = _score_groups(tkp)
                        sp = ps_s.tile([P, len(groups), 512], F32, tag="sp")
                        gs = 0
                        for gi, gn in enumerate(groups):
                            for k in range(KT):
                                nc.tensor.matmul(
                                    sp[:, gi, 0:gn],
                                    hT_sb[:, k, c * P:(c + 1) * P],
                                    zT_sb[:, k, gs:gs + gn],
                                    start=(k == 0),
                                    stop=(k == KT - 1),
                                )
                            gs += gn
                        sc_sb = chk.tile([P, tkp], F32, tag="sc")
                        gs = 0
                        for gi, gn in enumerate(groups):
                            nc.vector.tensor_add(
                                sc_sb[:, gs:gs + gn],
                                sp[:, gi, 0:gn],
                                mask_sb[:, gs:gs + gn],
                            )
                            gs += gn
                        negmax = stat.tile([P, 1], F32, tag="negmax")
                        nc.vector.reduce_max(
                            negmax, sc_sb, axis=mybir.AxisListType.X, negate=True
                        )
                        w_new = chk.tile([P, tkp], BF16, tag="w")
                        ssum = stat.tile([P, 1], F32, tag="ssum")
                        nc.scalar.activation(
                            w_new, sc_sb, mybir.ActivationFunctionType.Exp,
                            bias=negmax, scale=1.0, accum_out=ssum,
                        )
                        rsum_new = stat.tile([P, 1], F32, tag="rsum")
                        nc.vector.reciprocal(rsum_new, ssum)

                    if carry is not None:
                        pc, w_t, rsum_t = carry
                        cxp = ps_c.tile([P, H], F32, tag="cx")
                        for n in range(H // 512):
                            for j in range(jt):
                                nc.tensor.matmul(
                                    cxp[:, n * 512:(n + 1) * 512],
                                    wT_sb[:, j * P:(j + 1) * P],
                                    xn_sb[:, j, n * 512:(n + 1) * 512],
                                    start=(j == 0),
                                    stop=(j == jt - 1),
                                )
                        outc = chk.tile([P, H], F32, tag="outc")
                        for n in range(H // 512):
                            nsl = slice(n * 512, (n + 1) * 512)
                            nc.vector.tensor_scalar_mul(
                                outc[:, nsl], cxp[:, nsl], rsum_t
                            )
                            nc.gpsimd.dma_start(
                                ctx_d[b, pc * P:(pc + 1) * P, nsl],
                                outc[:, nsl],
                            )

                    carry = (c, w_new, rsum_new) if c < CH else None
    return nc


_CACHE = {}


def _get_nc(tkp0, tkp1):
    key = (tkp0, tkp1)
    if key not in _CACHE:
        nc = _build_nc(tkp0, tkp1)
        nc.compile()
        _CACHE[key] = nc
    return _CACHE[key]


def kernel(h_t_dec, x_enc, mask, W, b, _trace=False, _trace_kwargs=None):
    import ml_dtypes

    h_t_dec = np.ascontiguousarray(h_t_dec, dtype=np.float32)
    x_enc = np.ascontiguousarray(x_enc, dtype=np.float32)
    mask = np.asarray(mask).astype(bool)
    W = np.ascontiguousarray(W, dtype=np.float32)
    b = np.ascontiguousarray(b, dtype=np.float32)

    # Wn[m, p, k, c] = W[k*128 + p, m*128 + c] (W natural: kxm for z = x@W)
    Wm = np.ascontiguousarray(
        W.reshape(KT, P, KT, P).transpose(2, 1, 0, 3).reshape(KT, P, H))

    keep = [np.nonzero(~mask[bi])[0] for bi in range(B)]

    def pad128(n):
        return min(TK, max(P, ((n + P - 1) // P) * P))

    # Load-balance: sort batches by unmasked count; slot 0 takes the 8
    # smallest, slot 1 the 8 largest, so each slot's compiled width covers
    # only its own worst case instead of the global max.
    order = np.argsort([len(k) for k in keep], kind="stable")
    slot_batches = [order[:NCORES], order[NCORES:]]        # [slot][core]
    tkp0 = pad128(max(len(keep[g]) for g in slot_batches[0]))
    tkp1 = pad128(max(len(keep[g]) for g in slot_batches[1]))
    tkpm = max(tkp0, tkp1)

    # compacted x at the max width: unmasked rows first, zero rows beyond
    xc = np.zeros((B, tkpm, H), dtype=np.float32)
    amask_full = np.full((B, tkpm), _MASK_NEG, dtype=np.float32)
    for bi in range(B):
        nk = len(keep[bi])
        xc[bi, :nk] = x_enc[bi, keep[bi]]
        # score = h @ (x@W).T + (x@b): fold the bias term into the mask row
        amask_full[bi, :nk] = (
            xc[bi, :nk].astype(np.float64) @ b.astype(np.float64)
        ).astype(np.float32)

    in_maps = []
    for core in range(NCORES):
        gb = [slot_batches[0][core], slot_batches[1][core]]
        hT = h_t_dec[gb].transpose(0, 2, 1)              # [b, H, Tq]
        hT_t = np.ascontiguousarray(
            hT.reshape(BPC, KT, P, NH, 512).transpose(0, 3, 1, 2, 4))
        xT = np.ascontiguousarray(xc[gb].transpose(0, 2, 1))
        xn = np.ascontiguousarray(xc[gb]).astype(ml_dtypes.bfloat16)
        in_maps.append({
            "hT": hT_t,
            "xT": xT,
            "xn": xn,
            "Wm": Wm,
            "amask": np.ascontiguousarray(amask_full[gb]),
        })

    nc = _get_nc(tkp0, tkp1)
    res = run_bass_kernel_spmd(
        nc, in_maps, core_ids=list(range(NCORES)),
        trace=_trace, trace_kwargs=_trace_kwargs or {},
    )
    out = np.empty((B, TQ, H), dtype=np.float32)
    for core in range(NCORES):
        out[slot_batches[0][core]] = res.results[core]["ctx"][0]
        out[slot_batches[1][core]] = res.results[core]["ctx"][1]
    if _trace:
        return out, res
    return out


# revision 29
# speedup vs baseline: 1.1559x; 1.0031x over previous
"""Luong attention (linear -> bmm -> mask -> softmax -> bmm) on 8 trn2 cores.

Reference (per batch b):
    q = h @ W.T + b                  [Tq, H]
    s = q @ x.T                      [Tq, Tk]
    s = where(mask, -inf, s)
    w = softmax(s, axis=-1)
    ctx = w @ x                      [Tq, H]

Sharding: pure data-parallel over B=16 -> 2 batches per core, no collectives.

Mask compaction: masked positions get softmax weight exactly 0, so the host
gathers only the unmasked rows of x (per batch), zero-padded to TKP (a
multiple of 128 covering the largest unmasked count across batches). Padded
columns carry an additive -1e30 mask row so exp() underflows to exactly 0,
and the padded x rows are zero so the context contribution is exactly 0.
This halves the score and context matmul work for ~50% dense masks while
remaining mathematically exact for ANY mask.

Projection on the compacted side: score = (h@W.T + b)@x.T re-associates to
h @ (x@W).T + (x@b), so the projection matmul z = x_compact @ W contracts
over the compacted width instead of full Tq, and the bias term x@b is folded
into the host-built additive mask row for free.

Per-core device pipeline (per batch):
    zT = W.T-tiles @ xT              fp32r matmuls over compacted width
    per 128-row chunk of Tq:
      score psum = sum_k hT_k.T @ zT_k            fp32r, bank-aligned groups
      masked = score + (maskrow = x@b | -1e30 pad) -> SBUF (DVE)
      negmax = -rowmax(masked); w = exp(masked - max) with accum row sums
      wT = PE-transpose(w) in bf16 via PSUM
      ctx psum = sum_j wT_j.T @ x_j               bf16
      out = ctx * (1/rowsum)                      DVE scale, DMA out (gpsimd)

Host pre-layout: W tiled m-major pre-packed (natural orientation), hT tiled
n-half-major, per-slot compacted xT/x widths load-balanced by sorting batches
on unmasked count (slot 0 = 8 smallest), x cast to bf16 for the context
matmul, mask rows broadcast across partitions by a 0-stride DMA. float32r
(TF32-like) runs the PE at full rate.
"""
import numpy as np

import concourse.bacc as bacc


def _install_ntff_hook_shim():
    """The agent image's `antenv` lacks `axon_hooks`; bass_utils imports it
    for trace=True under axon. Provide it and register the ctypes hook."""
    import sys
    import types
    try:
        import antenv.axon_hooks  # noqa: F401
        return
    except ImportError:
        pass
    mod = types.ModuleType("antenv.axon_hooks")
    _state = {"hook": None}
    mod.set_axon_ntff_profile_hook = lambda h: _state.__setitem__("hook", h)
    mod.get_axon_ntff_profile_hook = lambda: _state["hook"]
    sys.modules["antenv.axon_hooks"] = mod
    try:
        import antenv
        antenv.axon_hooks = mod
    except ImportError:
        pass
    try:
        from trn_agent_boot.trn_boot import _ntff_profile_via_ctypes
        hook = _ntff_profile_via_ctypes("/opt/axon/libaxon_pjrt.so")
        if hook is not None:
            mod.set_axon_ntff_profile_hook(hook)
    except Exception:
        pass


_install_ntff_hook_shim()

import concourse.mybir as mybir  # noqa: E402
import concourse.tile as tile  # noqa: E402
from concourse.bass_utils import run_bass_kernel_spmd  # noqa: E402

F32 = mybir.dt.float32
F32R = mybir.dt.float32r
BF16 = mybir.dt.bfloat16

B, TQ, TK, H = 16, 1024, 1024, 1024
NCORES = 8
BPC = B // NCORES          # batches per core
P = 128
KT = H // P                # 8 k-tiles of the contraction dims
CH = TQ // P               # 8 q-row chunks per batch
NH = TQ // 512             # q-phase free-dim halves

_MASK_NEG = np.float32(-1e30)


def _score_groups(tkp):
    """Bank-packed group sizes: one group per PSUM bank, each >=256 where
    possible (avoids any small-N fp32r edge) and never spanning a bank."""
    rem, groups = tkp, []
    while rem > 0:
        if rem > 512:
            g = 512 if rem - 512 >= 256 else rem - 256
        else:
            g = rem
        groups.append(g)
        rem -= g
    return groups


def _build_nc(tkp0, tkp1):
    tkpm = max(tkp0, tkp1)     # DRAM tensors sized to the wider slot
    nc = bacc.Bacc("TRN2", target_bir_lowering=False)
    # Wm: [m, 128, H] m-major column tiles of W.T, pre-packed [p, k, c]
    Wm_d = nc.dram_tensor("Wm", [KT, P, H], F32R, kind="ExternalInput")
    # hT tiled [b, n, k, 128, 512]
    hT_d = nc.dram_tensor("hT", [BPC, NH, KT, P, 512], F32R, kind="ExternalInput")
    xT_d = nc.dram_tensor("xT", [BPC, H, tkpm], F32R, kind="ExternalInput")
    xn_d = nc.dram_tensor("xn", [BPC, tkpm, H], BF16, kind="ExternalInput")
    am_d = nc.dram_tensor("amask", [BPC, tkpm], F32, kind="ExternalInput")
    ctx_d = nc.dram_tensor("ctx", [BPC, TQ, H], F32, kind="ExternalOutput")

    with tile.TileContext(nc) as tc:
        with (
            tc.tile_pool(name="consts", bufs=1) as consts,
            tc.tile_pool(name="hTp", bufs=2) as hT_pool,
            tc.tile_pool(name="qTp", bufs=1) as qT_pool,
            tc.tile_pool(name="xTp", bufs=1) as xT_pool,
            tc.tile_pool(name="xnp", bufs=1) as xn_pool,
            tc.tile_pool(name="chk", bufs=2) as chk,
            tc.tile_pool(name="stat", bufs=2) as stat,
            tc.tile_pool(name="ps_s", bufs=2, space="PSUM") as ps_s,
            tc.tile_pool(name="ps_w", bufs=1, space="PSUM") as ps_w,
            tc.tile_pool(name="ps_c", bufs=1, space="PSUM") as ps_c,
        ):
            from concourse.masks import make_identity
            ident = consts.tile([P, P], BF16, tag="ident")
            make_identity(nc, ident)
            w_sb = consts.tile([P, KT, KT, P], F32R, tag="W")  # [p, m, k, c]

            def dma_w(m):
                nc.scalar.dma_start(
                    w_sb[:, m],
                    Wm_d[m].rearrange("p (k c) -> p k c", k=KT),
                )

            for b in range(BPC):
                tkp = (tkp0, tkp1)[b]   # per-slot compacted width
                jt = tkp // P
                # ---- input DMAs, first-needed-first: the projection
                # z = x@W needs xT + W; hT is only the score stationary ----
                xT_sb = xT_pool.tile([P, KT, tkp], F32R, tag="xT")
                for k in range(KT):
                    nc.sync.dma_start(
                        xT_sb[:, k], xT_d[b, k * P:(k + 1) * P, 0:tkp])
                if b == 0:
                    for m in range(KT):
                        dma_w(m)
                hT_sb = hT_pool.tile([P, KT, TQ], F32R, tag="hT")
                for n in range(NH):
                    for k in range(KT):
                        nc.sync.dma_start(
                            hT_sb[:, k, n * 512:(n + 1) * 512], hT_d[b, n, k]
                        )
                mask_sb = stat.tile([P, tkp], F32, tag="maskb")
                nc.sync.dma_start(
                    mask_sb, am_d[b:b + 1, 0:tkp].partition_broadcast(P)
                )
                xn_sb = xn_pool.tile([P, jt, H], BF16, tag="xn")
                for j in range(jt):
                    nc.sync.dma_start(xn_sb[:, j], xn_d[b, j * P:(j + 1) * P, :])

                # ---- projection over the COMPACTED width:
                # zT[m][i, s] = sum_k Wn_k,m.T @ xT_k  (z = x @ W) ----
                zT_sb = qT_pool.tile([P, KT, tkp], F32R, tag="qT")
                for m in range(KT):
                    gs = 0
                    for gn in _score_groups(tkp):
                        zp = ps_s.tile([P, 512], F32, tag="sp")
                        for k in range(KT):
                            nc.tensor.matmul(
                                zp[:, 0:gn],
                                w_sb[:, m, k],
                                xT_sb[:, k, gs:gs + gn],
                                start=(k == 0),
                                stop=(k == KT - 1),
                            )
                        nc.vector.tensor_copy(
                            zT_sb[:, m, gs:gs + gn], zp[:, 0:gn])
                        gs += gn

                # ---- chunk pipeline over Tq rows ----
                carry = None
                for c in range(CH + 1):
                    if carry is not None:
                        pc, w_t, rsum_t = carry
                        wTp = ps_w.tile([P, tkp], BF16, tag="wt")
                        for j in range(jt):
                            nc.tensor.transpose(
                                wTp[:, j * P:(j + 1) * P],
                                w_t[:, j * P:(j + 1) * P],
                                ident,
                            )
                        wT_sb = chk.tile([P, tkp], BF16, tag="wT")
                        nc.vector.tensor_copy(wT_sb, wTp)

                    if c < CH:
                        groups = _score_groups(tkp)
                        sp = ps_s.tile([P, len(groups), 512], F32, tag="sp")
                        gs = 0
                        for gi, gn in enumerate(groups):
                            for k in range(KT):
                                nc.tensor.matmul(
                                    sp[:, gi, 0:gn],
                                    hT_sb[:, k, c * P:(c + 1) * P],
                                    zT_sb[:, k, gs:gs + gn],
                                    start=(k == 0),
                                    stop=(k == KT - 1),
                                )
                            gs += gn
                        sc_sb = chk.tile([P, tkp], F32, tag="sc")
                        gs = 0
                        for gi, gn in enumerate(groups):
                            nc.vector.tensor_add(
                                sc_sb[:, gs:gs + gn],
                                sp[:, gi, 0:gn],
                                mask_sb[:, gs:gs + gn],
                            )
                            gs += gn
                        negmax = stat.tile([P, 1], F32, tag="negmax")
                        nc.vector.reduce_max(
                            negmax, sc_sb, axis=mybir.AxisListType.X, negate=True
                        )
                        w_new = chk.tile([P, tkp], BF16, tag="w")
                        ssum = stat.tile([P, 1], F32, tag="ssum")
                        nc.scalar.activation(
                            w_new, sc_sb, mybir.ActivationFunctionType.Exp,
                            bias=negmax, scale=1.0, accum_out=ssum,
                        )
                        rsum_new = stat.tile([P, 1], F32, tag="rsum")
                        nc.vector.reciprocal(rsum_new, ssum)

                    if carry is not None:
                        pc, w_t, rsum_t = carry
                        cxp = ps_c.tile([P, H], F32, tag="cx")
                        for n in range(H // 512):
                            for j in range(jt):
                                nc.tensor.matmul(
                                    cxp[:, n * 512:(n + 1) * 512],
                                    wT_sb[:, j * P:(j + 1) * P],
                                    xn_sb[:, j, n * 512:(n + 1) * 512],
                                    start=(j == 0),
                                    stop=(j == jt - 1),
                                )
                        outc = chk.tile([P, H], F32, tag="outc")
                        for n in range(H // 512):
                            nsl = slice(n * 512, (n + 1) * 512)
                            nc.scalar.activation(
                                outc[:, nsl], cxp[:, nsl],
                                mybir.ActivationFunctionType.Copy,
                                scale=rsum_t,
                            )
                            nc.gpsimd.dma_start(
                                ctx_d[b, pc * P:(pc + 1) * P, nsl],
                                outc[:, nsl],
                            )

                    carry = (c, w_new, rsum_new) if c < CH else None
    return nc


_CACHE = {}


def _get_nc(tkp0, tkp1):
    key = (tkp0, tkp1)
    if key not in _CACHE:
        nc = _build_nc(tkp0, tkp1)
        nc.compile()
        _CACHE[key] = nc
    return _CACHE[key]


def kernel(h_t_dec, x_enc, mask, W, b, _trace=False, _trace_kwargs=None):
    import ml_dtypes

    h_t_dec = np.ascontiguousarray(h_t_dec, dtype=np.float32)
    x_enc = np.ascontiguousarray(x_enc, dtype=np.float32)
    mask = np.asarray(mask).astype(bool)
    W = np.ascontiguousarray(W, dtype=np.float32)
    b = np.ascontiguousarray(b, dtype=np.float32)

    # Wn[m, p, k, c] = W[k*128 + p, m*128 + c] (W natural: kxm for z = x@W)
    Wm = np.ascontiguousarray(
        W.reshape(KT, P, KT, P).transpose(2, 1, 0, 3).reshape(KT, P, H))

    keep = [np.nonzero(~mask[bi])[0] for bi in range(B)]

    def pad128(n):
        return min(TK, max(P, ((n + P - 1) // P) * P))

    # Load-balance: sort batches by unmasked count; slot 0 takes the 8
    # smallest, slot 1 the 8 largest, so each slot's compiled width covers
    # only its own worst case instead of the global max.
    order = np.argsort([len(k) for k in keep], kind="stable")
    slot_batches = [order[:NCORES], order[NCORES:]]        # [slot][core]
    tkp0 = pad128(max(len(keep[g]) for g in slot_batches[0]))
    tkp1 = pad128(max(len(keep[g]) for g in slot_batches[1]))
    tkpm = max(tkp0, tkp1)

    # compacted x at the max width: unmasked rows first, zero rows beyond
    xc = np.zeros((B, tkpm, H), dtype=np.float32)
    amask_full = np.full((B, tkpm), _MASK_NEG, dtype=np.float32)
    for bi in range(B):
        nk = len(keep[bi])
        xc[bi, :nk] = x_enc[bi, keep[bi]]
        # score = h @ (x@W).T + (x@b): fold the bias term into the mask row
        amask_full[bi, :nk] = (
            xc[bi, :nk].astype(np.float64) @ b.astype(np.float64)
        ).astype(np.float32)

    in_maps = []
    for core in range(NCORES):
        gb = [slot_batches[0][core], slot_batches[1][core]]
        hT = h_t_dec[gb].transpose(0, 2, 1)              # [b, H, Tq]
        hT_t = np.ascontiguousarray(
            hT.reshape(BPC, KT, P, NH, 512).transpose(0, 3, 1, 2, 4))
        xT = np.ascontiguousarray(xc[gb].transpose(0, 2, 1))
        xn = np.ascontiguousarray(xc[gb]).astype(ml_dtypes.bfloat16)
        in_maps.append({
            "hT": hT_t,
            "xT": xT,
            "xn": xn,
            "Wm": Wm,
            "amask": np.ascontiguousarray(amask_full[gb]),
        })

    nc = _get_nc(tkp0, tkp1)
    res = run_bass_kernel_spmd(
        nc, in_maps, core_ids=list(range(NCORES)),
        trace=_trace, trace_kwargs=_trace_kwargs or {},
    )
    out = np.empty((B, TQ, H), dtype=np.float32)
    for core in range(NCORES):
        out[slot_batches[0][core]] = res.results[core]["ctx"][0]
        out[slot_batches[1][core]] = res.results[core]["ctx"][1]
    if _trace:
        return out, res
    return out
